# revision 29
# baseline (speedup 1.0000x reference)
"""Walsh-Hadamard transform (4096-point, orthonormal) on trn2, 8 cores.

y[r] = (H_4096 @ x[r]) / 64  for each of 16384 rows.

Scheme: H_4096 = H_8 (x) H_4 (x) H_128 over n = i*512 + v*128 + u
(i in 8, v in 4, u in 128). Rows are processed in groups of 16. An SBUF
tile holds a 16-row group as [128 partitions = (rr*8 + i), 512 free =
(v,u)]; each partition row is one contiguous 2 KiB chunk of DRAM (512
f32), which keeps DMA descriptors at full fabric-side efficiency.

Compute is in bf16 (the f32->bf16 cast happens inline in the SDMA
engines during both the load and the store, via SWDGE cast-DMA; all
Hadamard factors are exactly representable: +-1 and +-2^-6),
accumulation in fp32 PSUM. Two matmul stages per group:
  mm1 (x4, one per v): out1_v = Xslice_v.T @ BD   (BD = I_16 (x) H_8)
      -> [u, (rr,a)] in PSUM; the data is the stationary operand so the
      matmul also performs the layout corner-turn.
  mm2 (x4, accumulating, N=512): ps2 += t1_v.T @ M_v with
      M_v[u, v'*128+u'] = H4[v',v] * H128[u,u'] / 64
      -> [(rr,a), (v',u')] which is exactly the natural row-major output
      layout, so the store is also plain 2 KiB-chunk DMAs.

Work is sharded row-wise: core c processes rows [c*2048, (c+1)*2048).
"""

import numpy as np

N_ROWS = 16384
DIM = 4096
N_CORES = 8
R_PER_CORE = N_ROWS // N_CORES  # 2048

G = 4  # 16-row groups per DMA chunk -> 64 rows = 1 MiB per direction

_PROG_CACHE = {}


def _hadamard(n: int) -> np.ndarray:
    H = np.array([[1.0]], dtype=np.float64)
    while H.shape[0] < n:
        H = np.block([[H, H], [H, -H]])
    return H


def _build_program():
    import concourse.mybir as mybir
    from concourse import bacc
    from concourse.tile import TileContext

    f32 = mybir.dt.float32
    bf16 = mybir.dt.bfloat16
    nc = bacc.Bacc("TRN2")

    x = nc.declare_dram_parameter("x", [R_PER_CORE, DIM], f32, isOutput=False)
    y = nc.declare_dram_parameter("y", [R_PER_CORE, DIM], f32, isOutput=True)

    BD = np.kron(np.eye(16), _hadamard(8)).astype(np.float32)  # [(rr,i),(rr,a)]
    Hs = _hadamard(128) / 64.0  # [u, u']
    H4 = _hadamard(4)  # [v', v]
    Ms = [
        np.concatenate([H4[vp, v] * Hs for vp in range(4)], axis=1).astype(
            np.float32
        )
        for v in range(4)
    ]

    bd_d = nc.inline_tensor(BD, "bd_const")
    m_d = [nc.inline_tensor(Ms[v], f"m{v}_const") for v in range(4)]

    n_chunks = R_PER_CORE // (16 * G)  # 32

    xv = x[:].rearrange("(cb g rr) (i jj) -> cb (rr i) g jj", g=G, rr=16, i=8, jj=512)
    yv = y[:].rearrange("(cb g rr) (a jj) -> cb (rr a) g jj", g=G, rr=16, a=8, jj=512)
    # Quarter-size views for the tail (drain shrink): 128 chunks of 16 rows.
    GT = 1
    xt = x[:].rearrange("(cb g rr) (i jj) -> cb (rr i) g jj", g=GT, rr=16, i=8, jj=512)
    yt = y[:].rearrange("(cb g rr) (a jj) -> cb (rr a) g jj", g=GT, rr=16, a=8, jj=512)

    with TileContext(nc) as tc:
        with (
            tc.tile_pool(name="consts", bufs=1) as cpool,
            tc.tile_pool(name="inbf", bufs=18) as bfpool,
            tc.tile_pool(name="outp", bufs=18) as outpool,
            tc.tile_pool(name="mid", bufs=12) as midpool,
            tc.tile_pool(name="tails", bufs=4) as tailpool,
            tc.tile_pool(name="ps1", bufs=4, space="PSUM") as ps1pool,
            tc.tile_pool(name="ps2", bufs=4, space="PSUM") as ps2pool,
        ):
            bd_f = cpool.tile([128, 128], f32)
            nc.sync.dma_start(out=bd_f[:], in_=bd_d[:])
            bd_sb = cpool.tile([128, 128], bf16)
            nc.vector.tensor_copy(out=bd_sb[:], in_=bd_f[:])
            m_sb = []
            for v in range(4):
                m_f = cpool.tile([128, 512], f32, tag=f"mf{v}")
                nc.sync.dma_start(out=m_f[:], in_=m_d[v][:])
                m_b = cpool.tile([128, 512], bf16, tag=f"mb{v}")
                nc.vector.tensor_copy(out=m_b[:], in_=m_f[:])
                m_sb.append(m_b)

            # Stores are issued 1 chunk late in program order so the Q7
            # SWDGE desc-gen never stalls waiting for compute (a stalled
            # store-gen would starve the load stream behind it).
            STORE_LAG = 1
            pending = []

            def compute_chunk(in_bf, out_tile, n_groups):
                for g in range(n_groups):
                    ps1 = ps1pool.tile([128, 512], f32)
                    for v in range(4):
                        nc.tensor.matmul(
                            ps1[:, v * 128 : (v + 1) * 128],
                            in_bf[:, g, v * 128 : (v + 1) * 128],
                            bd_sb[:],
                            start=True,
                            stop=True,
                        )
                    t1 = midpool.tile([128, 512], bf16)
                    nc.scalar.copy(t1[:], ps1[:])
                    ps2 = ps2pool.tile([128, 512], f32)
                    for v in range(4):
                        nc.tensor.matmul(
                            ps2[:],
                            t1[:, v * 128 : (v + 1) * 128],
                            m_sb[v][:],
                            start=(v == 0),
                            stop=(v == 3),
                        )
                    nc.vector.tensor_copy(out=out_tile[:, g], in_=ps2[:])

            for cb in range(n_chunks - 1):
                # SWDGE cast-during-DMA: reads f32 from HBM, lands bf16
                # in SBUF (the cast runs inline in the SDMA engines).
                in_bf = bfpool.tile([128, G, 512], bf16)
                nc.gpsimd.dma_start(out=in_bf[:], in_=xv[cb])
                out_tile = outpool.tile([128, G, 512], bf16)
                compute_chunk(in_bf, out_tile, G)
                # SWDGE cast-during-DMA on the store too: bf16 in SBUF,
                # f32 in HBM (halves the SBUF-side S2M traffic).
                pending.append((yv[cb], out_tile))
                if len(pending) > STORE_LAG:
                    sap, stile = pending.pop(0)
                    nc.gpsimd.dma_start(out=sap, in_=stile[:])
            # Tail: run the last chunk as 4 quarter-chunks so the pipeline
            # drain (last compute + last store) is short.
            n_tail = G // GT
            for t in range(n_tail):
                tcb = (n_chunks - 1) * n_tail + t
                in_bf = tailpool.tile([128, GT, 512], bf16, tag="tailin")
                nc.gpsimd.dma_start(out=in_bf[:], in_=xt[tcb])
                out_tile = tailpool.tile([128, GT, 512], bf16, tag="tailout")
                compute_chunk(in_bf, out_tile, GT)
                pending.append((yt[tcb], out_tile))
            for sap, stile in pending:
                nc.gpsimd.dma_start(out=sap, in_=stile[:])

    nc.compile()
    return nc


def _get_program():
    if "nc" not in _PROG_CACHE:
        _PROG_CACHE["nc"] = _build_program()
    return _PROG_CACHE["nc"]


def kernel(x, _trace=False, _trace_kwargs=None):
    from concourse.bass_utils import run_bass_kernel_spmd

    x = np.ascontiguousarray(np.asarray(x, dtype=np.float32))
    assert x.shape == (N_ROWS, DIM), x.shape

    nc = _get_program()
    core_ids = list(range(N_CORES))
    in_maps = [
        {"x": x[c * R_PER_CORE : (c + 1) * R_PER_CORE]} for c in core_ids
    ]
    res = run_bass_kernel_spmd(
        nc, in_maps, core_ids, trace=_trace, **(_trace_kwargs or {})
    )
    out = np.concatenate([r["y"] for r in res.results], axis=0)
    if _trace:
        return out, res
    return out
